# revision 12
# baseline (speedup 1.0000x reference)
"""Trainium2 Bass kernel for 3D multi-head attention (nn_Attention3D).

Problem: x [1, 16, 16, 16, 528] -> full attention over N=4096 tokens,
8 heads of dim 66, qkv + out projections.

Sharding: one head per NeuronCore (8 cores). Each core computes its
head's q/k/v projections, full 4096x4096 attention, and its partial
contribution to the output projection. Host sums the 8 partials and
adds the output bias.

Key layout decisions (all host-side prep, free):
  - x is pre-transposed on host to xT [640, 4096] (C on partitions),
    with row 528 = 1.0 (bias row) and rows 529-639 = 0 padding; qkv
    biases fold into the weight matmuls via the ones row.
  - Scores are computed transposed ([k-tokens, q-tokens]) so softmax's
    sum runs over the partition dim via a ones-column in the v weights
    (the attention-value matmul accumulates the denominator for free),
    and no transposes are ever needed.
  - Scores are produced directly in "exp2-bits" units: hd^-0.5 * log2e
    * 2^23 is folded into wq/bq, and a global -0.5*2^23 shift rides on
    the spare qT/kT row 66 (a constant shift of all scores cancels in
    softmax normalization).  This lets the exp over the 16.7M-score
    matrix be SPLIT across two engines:
      * ScalarE: native Exp activation with scale=ln2*2^-23,
        bias=+0.5*ln2 in the free affine -> bf16 E.
      * VectorE: a one-instruction custom DVE op that assembles the
        fp32 BIT PATTERN of 2^t arithmetically (magic-add floor range
        reduction, quadratic mantissa polynomial, +(127+a0)*2^23) and
        writes through an int32-convert output.  The attention-value
        matmul then reads the high 2 bytes of each fp32 via a stride-2
        bf16 bitcast AP (truncation bias cancels in the softmax
        normalization since the denominator sees the same values).
    Baseline had exp 100% on ScalarE (140us, co-critical with the PE);
    the split makes phase B purely PE-bound.
  - All matmuls are bf16 (1 cyc/row + fast weight load), including the
    output projection (baseline used float32r there: ~660ns/MM vs
    ~213ns bf16).  PSUM accumulation is always fp32.  y partials are
    DMA'd out as bf16 and summed in fp32 on host.
"""

import numpy as np

import ml_dtypes

BF16_NP = ml_dtypes.bfloat16

EMBED = 528
HD = 66
NHEADS = 8
NT = 4096
NCH = 5  # contraction chunks of 128 (640 = 528 + bias row + pad)

# exp2-bits constants (see exp_dve.py experiment)
MAGIC = 1.5 * 2.0**46
A0 = 0.414839277933763
A1 = 0.9948016962806719
A2 = 0.33717699739561857
BIAS_E = (127.0 + A0) * 2.0**23
LN2 = float(np.log(2.0))
LN2_SC = LN2 / 2.0**23
C_SCORE = float(HD**-0.5 / LN2 * 2.0**23)  # fold into wq/bq
SHIFT_K = -0.5 * 2.0**23  # qT/kT row 66 rank-1 constant shift


def _register_exp2():
    """Register the one-instruction DVE 2^x-bits op (idempotent)."""
    import concourse.dve_ops as dve_ops

    for op in dve_ops.OPS:
        if op.name == "EXP2_BITS_ANT":
            return op

    from concourse.dve_spec import (
        Spec,
        Src0,
        C0,
        C1,
        C2,
        C3,
        lower,
        _spill_c3_to_src1,
    )
    from concourse.dve_uop import DveOpSpec

    # Input is ts23 = (t - 0.5)*2^23; out_bits = fp32 bits of 2^t:
    #   p   = ts23 + 1.5*2^46  (rounds ts23 to nearest multiple of 2^23)
    #   i23 = p - 1.5*2^46     (= floor(t)*2^23 exact)
    #   X   = ts23 - i23       (= (f-0.5)*2^23, f = t-floor(t), exact)
    #   m   = X*(a1 + (a2/2^23)*X)
    #   y   = m + i23 + (127 + a0)*2^23
    # with a0 + a1 x + a2 x^2 ~= sqrt2*2^x - 1 = 2^f - 1 >= 0 on x in
    # [-0.5, 0.5]; int32-convert writeback turns the value into bits.
    p = Src0 + C0
    i23 = p - C0
    x = Src0 - i23
    m = (x * C3 + C1) * x
    y = (m + i23) + C2

    def _ref(in0, in1, c0, c1, c2):
        f32 = np.float32
        t = in0.astype(f32)
        pp = (t + f32(c0)).astype(f32)
        ii = (pp - f32(c0)).astype(f32)
        xx = (t - ii).astype(f32)
        mm = (xx * ((in1 * xx).astype(f32) + f32(c1))).astype(f32)
        return (mm + ii + f32(c2)).astype(f32)

    spec = Spec(body=_spill_c3_to_src1(y), reference=_ref)
    shas = {}
    for ver in ("v3", "v4"):
        try:
            s = DveOpSpec(
                name="EXP2_BITS_ANT", opcode=None, uops=lower(spec, ver=ver),
                rd1_en=True,
            )
            shas[ver] = s.sha(ver)
        except Exception:
            pass
    op = dve_ops.DveOp("EXP2_BITS_ANT", spec, subdim=False, uops_sha=shas)
    dve_ops.OPS.append(op)
    dve_ops._SUB_OPCODE_FOR_NAME[op.name] = (
        dve_ops._CUSTOM_DVE_ROW_BASE + len(dve_ops.OPS) - 1
    )
    dve_ops.CUSTOM_DVE_SPECS[op.name] = op.spec
    return op


def _build_nc(nt=NT):
    import concourse.tile as tile
    from concourse import bacc, mybir

    exp2_op = _register_exp2()

    F32 = mybir.dt.float32
    I32 = mybir.dt.int32
    BF16 = mybir.dt.bfloat16
    AF = mybir.ActivationFunctionType

    nkc = nt // 128  # k-token chunks
    nqb = nt // 512  # q-token blocks
    ntb = nt // 128  # token blocks for the projection

    nc = bacc.Bacc("TRN2", target_bir_lowering=False, debug=False)
    xT_d = nc.dram_tensor("xT", [128, NCH, nt], BF16, kind="ExternalInput").ap()
    wq_d = nc.dram_tensor("wq", [128, NCH, 128], BF16, kind="ExternalInput").ap()
    wk_d = nc.dram_tensor("wk", [128, NCH, 128], BF16, kind="ExternalInput").ap()
    wv_d = nc.dram_tensor("wv", [128, NCH, HD + 2], BF16, kind="ExternalInput").ap()
    wp_d = nc.dram_tensor("wp", [128, EMBED], BF16, kind="ExternalInput").ap()
    cs_d = nc.dram_tensor("cs", [128, 2], F32, kind="ExternalInput").ap()
    y_d = nc.dram_tensor("y", [nt, EMBED], BF16, kind="ExternalOutput").ap()

    with tile.TileContext(nc) as tc:
        with (
            tc.tile_pool(name="const", bufs=1) as constp,
            tc.tile_pool(name="persist", bufs=1) as pp,
        ):
            wq = constp.tile([128, NCH, 128], BF16, name="wq_sb")
            wk = constp.tile([128, NCH, 128], BF16, name="wk_sb")
            wv = constp.tile([128, NCH, HD + 2], BF16, name="wv_sb")
            wp = constp.tile([128, EMBED], BF16, name="wp_sb")
            cs = constp.tile([128, 2], F32, name="cs_sb")
            warm = constp.tile([128, 1], F32, name="warm_sb")
            nc.sync.dma_start(cs[:], cs_d[:])
            c3_ap = cs[:, 0:1]
            cb_ap = cs[:, 1:2]

            # preload the exp table set early (2.7us, overlaps phase A)
            nc.scalar.activation(warm[:], cs[:, 0:1], AF.Exp, scale=0.0)

            qT = pp.tile([128, nt], BF16, name="qT")
            kT = pp.tile([128, nt], BF16, name="kT")
            oT = pp.tile([128, nt], BF16, name="oT")
            vaug = pp.tile([128, nkc, HD + 2], BF16, name="vaug")
            recipT = pp.tile([128, ntb], F32, name="recipT")
            # junk rows 68-127 of oT feed the projection lhsT; NaN*0 = NaN.
            # (partition bases must be 32-aligned, so clear all of oT; the
            # live rows 0-67 are overwritten per q-block in phase B)
            nc.gpsimd.memset(oT[:, :], 0.0)

            # ---------------- Phase A: qkv projections ----------------
            with (
                tc.tile_pool(name="xp", bufs=1) as xp,
                tc.tile_pool(name="psA", bufs=4, space="PSUM") as psA,
            ):
                xT = xp.tile([128, NCH, nt], BF16, name="xT_sb")
                # two HW DMA queues (sync + scalar), ~600ns trigger cost
                # each: block 0 lands chunk-by-chunk on sync so the first
                # matmul starts ~9us in; everything else is block-sized.
                b0 = slice(0, 512)
                for c in range(NCH):
                    nc.sync.dma_start(xT[:, c, b0], xT_d[:, c, b0])
                nc.scalar.dma_start(wq[:], wq_d[:])
                nc.scalar.dma_start(wk[:], wk_d[:])
                for b in range(1, nqb):
                    qs = slice(b * 512, (b + 1) * 512)
                    eng = nc.scalar if b % 2 else nc.sync
                    eng.dma_start(xT[:, :, qs], xT_d[:, :, qs])
                nc.scalar.dma_start(wv[:], wv_d[:])
                nc.sync.dma_start(wp[:], wp_d[:])

                # interleave the q and k accumulation chains (independent
                # PSUM banks) so consecutive PE matmuls pipeline
                for b in range(nqb):
                    qs = slice(b * 512, (b + 1) * 512)
                    ps_q = psA.tile([128, 512], F32, tag="qk", name="ps_q")
                    ps_k = psA.tile([128, 512], F32, tag="qk", name="ps_k")
                    for c in range(NCH):
                        for w, ps in ((wq, ps_q), (wk, ps_k)):
                            nc.tensor.matmul(
                                ps[:],
                                w[:, c, :],
                                xT[:, c, qs],
                                start=(c == 0),
                                stop=(c == NCH - 1),
                            )
                    nc.vector.tensor_copy(qT[:, qs], ps_q[:])
                    nc.vector.tensor_copy(kT[:, qs], ps_k[:])
                # v: two token-block chains in flight; copies on ScalarE
                # (VectorE owns the qT/kT casts, ScalarE is idle here)
                for t0 in range(0, nkc, 2):
                    psvs = [
                        psA.tile([128, HD + 2], F32, tag="v", name="ps_v")
                        for _ in range(2)
                    ]
                    for c in range(NCH):
                        for i in range(2):
                            ts_ = slice((t0 + i) * 128, (t0 + i + 1) * 128)
                            nc.tensor.matmul(
                                psvs[i][:],
                                xT[:, c, ts_],
                                wv[:, c, :],
                                start=(c == 0),
                                stop=(c == NCH - 1),
                            )
                    for i in range(2):
                        nc.scalar.activation(
                            vaug[:, t0 + i, :], psvs[i][:], AF.Copy
                        )

            # ---------------- Phase B: attention ----------------
            # 2-chunk score groups, triple-buffered (3x2 PSUM banks) + 1
            # bank for the oT accumulator = 7 of 8.  Per group, chunk 0's
            # exp runs on ScalarE (bf16 E) and chunk 1's on VectorE
            # (exp2-bits int32) -- different PSUM banks, so the reads are
            # parallel.  AV(g) is emitted after scores(g+2), giving the
            # exps ~1.3us of PE-time cover (one-group depth measurably
            # stalled the PE ~0.5us/group waiting on E).
            with (
                tc.tile_pool(name="eps", bufs=4) as eps,
                tc.tile_pool(name="edp", bufs=4) as edp,
                tc.tile_pool(name="rp", bufs=2) as rp,
                tc.tile_pool(name="drp", bufs=2, space="DRAM") as drp,
                tc.tile_pool(name="psS", bufs=3, space="PSUM") as psS,
                tc.tile_pool(name="psO", bufs=1, space="PSUM") as psO,
            ):
                for b in range(nqb):
                    qs = slice(b * 512, (b + 1) * 512)
                    o_ps = psO.tile([HD + 2, 512], F32, name="o_ps")

                    def emit_av(g0, Es, Ed):
                        ed_hi = (
                            Ed[:]
                            .bitcast(BF16)
                            .rearrange("p (n two) -> p n two", two=2)
                        )
                        for j in range(2):
                            kc = g0 + j
                            rhs = Es[:] if j == 0 else ed_hi[:, :, 1]
                            nc.tensor.matmul(
                                o_ps[:],
                                vaug[:, kc, :],
                                rhs,
                                start=(kc == 0),
                                stop=(kc == nkc - 1),
                                skip_group_check=True,
                            )

                    pend = []
                    for g in range(nkc // 2):
                        g0 = 2 * g
                        sc = psS.tile([128, 2, 512], F32, tag="sc", name="sc")
                        for j in range(2):
                            kc = g0 + j
                            nc.tensor.matmul(
                                sc[:, j, :],
                                kT[:, kc * 128 : (kc + 1) * 128],
                                qT[:, qs],
                                start=True,
                                stop=True,
                            )
                        Es = eps.tile([128, 512], BF16, tag="Es", name="Es")
                        Ed = edp.tile([128, 512], I32, tag="Ed", name="Ed")
                        nc.scalar.activation(
                            Es[:], sc[:, 0, :], AF.Exp, scale=LN2_SC, bias=cb_ap
                        )
                        nc.vector._custom_dve(
                            exp2_op,
                            out=Ed[:],
                            in0=sc[:, 1, :],
                            in1=c3_ap,
                            s0=MAGIC,
                            s1=A1,
                            imm2=BIAS_E,
                        )
                        pend.append((g0, Es, Ed))
                        if len(pend) > 2:
                            emit_av(*pend.pop(0))
                    for p_ in pend:
                        emit_av(*p_)
                    recip = rp.tile([1, 512], F32, name="recip")
                    nc.vector.reciprocal_approx_fast(recip[:], o_ps[0:1, :])
                    dstage = drp.tile([1, 512], F32, name="dstage")
                    nc.sync.dma_start(dstage[:], recip[:])
                    nc.sync.dma_start(
                        recipT[:, b * 4 : (b + 1) * 4],
                        dstage.rearrange("o (f p) -> (o p) f", p=128),
                    )
                    if b % 2:
                        nc.scalar.activation(oT[: HD + 2, qs], o_ps[:], AF.Copy)
                    else:
                        nc.vector.tensor_copy(oT[: HD + 2, qs], o_ps[:])

            # ---------------- Phase C: output projection ----------------
            with (
                tc.tile_pool(name="yp", bufs=4) as yp,
                tc.tile_pool(name="psY", bufs=4, space="PSUM") as psY,
            ):
                half = EMBED // 2  # 264
                for t in range(ntb):
                    ts_ = slice(t * 128, (t + 1) * 128)
                    yps = psY.tile([128, 2, 512], F32, name="yps")
                    nc.tensor.matmul(
                        yps[:, 0, :half],
                        oT[:, ts_],
                        wp[:, :half],
                        start=True,
                        stop=True,
                    )
                    nc.tensor.matmul(
                        yps[:, 1, :half],
                        oT[:, ts_],
                        wp[:, half:],
                        start=True,
                        stop=True,
                    )
                    ysb = yp.tile([128, 2, half], BF16, tag="ysb", name="ysb")
                    if t % 2 == 0:
                        nc.vector.tensor_scalar_mul(
                            ysb[:], yps[:, :, :half], recipT[:, t : t + 1]
                        )
                    else:
                        nc.scalar.activation(
                            ysb[:],
                            yps[:, :, :half],
                            AF.Copy,
                            scale=recipT[:, t : t + 1],
                        )
                    # y DMAs ride sync/gpsimd queues: a trigger on the scalar
                    # queue would serialize with the scale ACTIVATEs
                    eng = nc.gpsimd if t % 2 else nc.sync
                    eng.dma_start(
                        y_d[ts_, :], ysb[:].rearrange("p g n -> p (g n)")
                    )

    nc.compile()
    return nc


def _prep_inputs(x, w_qkv, b_qkv, w_proj, nt):
    """Host-side shard prep: returns list of 8 in_maps."""
    x = np.asarray(x, dtype=np.float32)
    w_qkv = np.asarray(w_qkv, dtype=np.float32)
    b_qkv = np.asarray(b_qkv, dtype=np.float32)
    w_proj = np.asarray(w_proj, dtype=np.float32)

    xt = x.reshape(nt, EMBED)
    xT_pad = np.zeros((NCH * 128, nt), dtype=np.float32)
    xT_pad[:EMBED] = xt.T
    xT_pad[EMBED] = 1.0
    # [128, NCH, nt]: partition-major so one DMA covers a token block
    xT_in = np.ascontiguousarray(
        xT_pad.reshape(NCH, 128, nt).transpose(1, 0, 2)
    ).astype(BF16_NP)

    cs = np.zeros((128, 2), dtype=np.float32)
    cs[:, 0] = A2 / 2.0**23
    cs[:, 1] = 0.5 * LN2

    in_maps = []
    for h in range(NHEADS):
        sl_q = slice(h * HD, (h + 1) * HD)
        sl_k = slice(EMBED + h * HD, EMBED + (h + 1) * HD)
        sl_v = slice(2 * EMBED + h * HD, 2 * EMBED + (h + 1) * HD)

        # q side carries the hd^-0.5 * log2e * 2^23 score scaling; the
        # spare column 66 carries the global -0.5*2^23 shift (qT66 = 1,
        # kT66 = -2^22) that the exp2-bits op's range reduction needs.
        wq_t = np.zeros((NCH * 128, 128), dtype=np.float32)
        wq_t[:EMBED, :HD] = (w_qkv[sl_q] * C_SCORE).T
        wq_t[EMBED, :HD] = b_qkv[sl_q] * C_SCORE
        wq_t[EMBED, HD] = 1.0

        wk_t = np.zeros((NCH * 128, 128), dtype=np.float32)
        wk_t[:EMBED, :HD] = w_qkv[sl_k].T
        wk_t[EMBED, :HD] = b_qkv[sl_k]
        wk_t[EMBED, HD] = SHIFT_K

        # ones column at index 0 so the softmax denominator lands on
        # PSUM partition 0 (engine partition bases must be 32-aligned)
        wv_t = np.zeros((NCH * 128, HD + 2), dtype=np.float32)
        wv_t[:EMBED, 1 : HD + 1] = w_qkv[sl_v].T
        wv_t[EMBED, 1 : HD + 1] = b_qkv[sl_v]
        wv_t[EMBED, 0] = 1.0  # ones column -> softmax denominator

        wp_t = np.zeros((128, EMBED), dtype=np.float32)
        wp_t[1 : HD + 1] = w_proj[:, sl_q].T  # row 0 = 0 kills the denom row

        in_maps.append(
            {
                "xT": xT_in,
                "wq": np.ascontiguousarray(
                    wq_t.reshape(NCH, 128, 128).transpose(1, 0, 2)
                ).astype(BF16_NP),
                "wk": np.ascontiguousarray(
                    wk_t.reshape(NCH, 128, 128).transpose(1, 0, 2)
                ).astype(BF16_NP),
                "wv": np.ascontiguousarray(
                    wv_t.reshape(NCH, 128, HD + 2).transpose(1, 0, 2)
                ).astype(BF16_NP),
                "wp": wp_t.astype(BF16_NP),
                "cs": cs,
            }
        )
    return in_maps


_NC_CACHE = {}


def _get_nc(nt=NT):
    if nt not in _NC_CACHE:
        _NC_CACHE[nt] = _build_nc(nt)
    return _NC_CACHE[nt]


def kernel(x, w_qkv, b_qkv, w_proj, b_proj, _trace=False):
    from concourse.bass_utils import run_bass_kernel_spmd

    x = np.asarray(x, dtype=np.float32)
    b_proj = np.asarray(b_proj, dtype=np.float32)
    B, D, H, W, C = x.shape
    nt = D * H * W

    nc = _get_nc(nt)
    in_maps = _prep_inputs(x, w_qkv, b_qkv, w_proj, nt)
    res = run_bass_kernel_spmd(
        nc, in_maps, core_ids=list(range(NHEADS)), trace=_trace
    )
    out = np.zeros((nt, EMBED), dtype=np.float32)
    for r in res.results:
        out += r["y"].astype(np.float32)
    out += b_proj
    kernel.last_results = res
    return out.reshape(B, D, H, W, C)
